# revision 1
# baseline (speedup 1.0000x reference)
"""Masked mean-pooling (nn_MaskedPooling) Trainium2 Bass kernel.

Reference semantics (jax):
    keep   = (~mask).astype(f32)               # [B, T]
    denom  = keep.sum(axis=1)                  # [B]
    out    = einsum('btd,bt->bd', x, keep) / denom[:, None]

Shapes: x [32, 4096, 512] f32, mask [32, 4096] bool -> out [32, 512] f32.

Strategy (data-parallel over batch, 8 NeuronCores, 4 examples/core):
  * T is split as t = p*32 + n (p = SBUF partition, n = chunk column), so
    every DMA reads one long contiguous run per partition (descriptor-
    light, full HBM bandwidth) and the keep matrix loads directly in the
    layout the PE needs -- no transpose anywhere.
  * The masked sum over T is a PE matmul: for each of the 32 T-chunks,
    the keep chunk ([128, 1] stationary operand) contracts with the x
    chunk [128, 512], accumulating over chunks in PSUM. The mask-weighted
    products are exact (keep is 0/1).
  * Denominators come from one matmul with a ones-vector against the
    keep matrix, then a free-dim reduce + reciprocal; the final scale is
    a per-example tensor_scalar on the PSUM accumulator.
  * x tiles stream via HWDGE DMAs alternating between the SP and ACT
    rings so consecutive transfers overlap their setup.

The kernel is memory-bound: 32 MiB of x per core at ~358 GB/s HBM ->
~90 us roofline.
"""

import os
from contextlib import ExitStack

import numpy as np

import concourse.bass as bass
import concourse.mybir as mybir
import concourse.tile as tile
from concourse import bacc, bass_utils

B, T, D = 32, 4096, 512
N_CORES = 8
BS = B // N_CORES  # examples per core
P = 128  # SBUF partitions
NCHUNK = T // P  # T-chunks per example (32)

# Tunables (overridable via env for A/B benchmarking from test.py).
CHUNKS_PER_TILE = int(os.environ.get("MP_CHUNKS_PER_TILE", "16"))  # 16 -> 4 MiB DMAs
X_BUFS = int(os.environ.get("MP_X_BUFS", "5"))
# "f32" (exact, PE 4 cyc/row) or "f32r" (single-pass fp32 matmul, PE 1 cyc/row)
MM_DTYPE = os.environ.get("MP_MM_DTYPE", "f32")
# 0=SWDGE (gpsimd, best measured for the fp32 config), 1=SP ring, 2=SP+ACT rings
N_DMA_ENGINES = int(os.environ.get("MP_DMA_ENGINES", "0"))


def build_bass(
    bs=BS,
    t=T,
    d=D,
    chunks_per_tile=CHUNKS_PER_TILE,
    x_bufs=X_BUFS,
    mm_dtype=MM_DTYPE,
    n_cores=N_CORES,
    n_dma_engines=N_DMA_ENGINES,
):
    nchunk = t // P
    assert t % P == 0 and nchunk % chunks_per_tile == 0
    # Bacc (not raw Bass): its compile() pass splits multi-semaphore waits
    # into event-semaphore chains — walrus accepts at most one sync wait
    # per instruction.
    nc = bacc.Bacc(
        trn_type="TRN2",
        target_bir_lowering=False,
        debug=False,
        num_devices=n_cores,
    )
    # float32r is bit-identical to float32 in memory (np maps it to
    # np.float32); declaring the tensors as f32r end-to-end satisfies the
    # BIR verifier's "producer must round to FP32r" rule with plain copies.
    mmdt = mybir.dt.float32r if mm_dtype == "f32r" else mybir.dt.float32
    x = nc.dram_tensor("x", [bs, t, d], mmdt, kind="ExternalInput").ap()
    mask = nc.dram_tensor("mask", [bs, t], mybir.dt.uint8, kind="ExternalInput").ap()
    out = nc.dram_tensor("out", [bs, d], mybir.dt.float32, kind="ExternalOutput").ap()

    with tile.TileContext(nc) as tc, ExitStack() as ctx:
        singles = ctx.enter_context(tc.tile_pool(name="singles", bufs=1))
        xpool = ctx.enter_context(tc.tile_pool(name="xpool", bufs=x_bufs))
        tails = ctx.enter_context(tc.tile_pool(name="tails", bufs=4))
        psum = ctx.enter_context(tc.tile_pool(name="psum", bufs=1, space="PSUM"))
        accs = ctx.enter_context(tc.tile_pool(name="accs", bufs=4, space="PSUM"))

        jcols = bs * nchunk  # one keep column per (example, T-chunk)
        assert jcols <= 512

        # ones vector for the denominator matmul.
        ones = singles.tile([P, 1], mmdt)
        if mmdt == mybir.dt.float32r:
            # Memset can't target f32r; produce via DVE copy (the "rounding"
            # producer the BIR verifier wants).
            ones_f32 = singles.tile([P, 1], mybir.dt.float32)
            nc.vector.memset(ones_f32, 1.0)
            nc.vector.tensor_copy(out=ones, in_=ones_f32)
        else:
            nc.vector.memset(ones, 1.0)

        # Mask loads directly in lhsT layout: m_u8[p, j] = mask[b, p*32 + n]
        # with j = b*nchunk + n (32 contiguous bytes per partition per
        # example).
        m_u8 = singles.tile([P, bs, nchunk], mybir.dt.uint8)
        nc.sync.dma_start(out=m_u8, in_=mask.rearrange("b (p n) -> p b n", p=P))
        m_f = singles.tile([P, bs, nchunk], mybir.dt.float32)
        nc.vector.tensor_copy(out=m_f, in_=m_u8)
        # keep = 1 - m
        keep = singles.tile([P, bs, nchunk], mmdt)
        nc.vector.tensor_scalar(
            out=keep,
            in0=m_f,
            scalar1=-1.0,
            scalar2=1.0,
            op0=mybir.AluOpType.mult,
            op1=mybir.AluOpType.add,
        )

        # Denominators: den[j] = sum_p keep[p, j]; reduce chunks per example.
        den_ps = psum.tile([1, bs, nchunk], mybir.dt.float32)
        nc.tensor.matmul(den_ps, ones, keep, start=True, stop=True)
        den = tails.tile([1, bs], mybir.dt.float32)
        nc.vector.tensor_reduce(
            out=den,
            in_=den_ps,
            axis=mybir.AxisListType.X,
            op=mybir.AluOpType.add,
        )
        rec = tails.tile([1, bs], mybir.dt.float32)
        nc.vector.reciprocal(rec, den)

        # 0 -> SWDGE (gpsimd); 1 -> SP ring; 2 -> alternate SP/ACT rings.
        # The tiny out-DMAs go on a DIFFERENT engine stream than the x
        # triggers: a shared FIFO would stall the next example's prefetch
        # behind the previous example's compute drain (measured 27 us
        # example-boundary bubbles).
        if n_dma_engines == 0:
            dma_engines = [nc.gpsimd]
            out_dma = nc.sync
        else:
            dma_engines = [nc.sync, nc.scalar][:n_dma_engines]
            out_dma = nc.gpsimd

        # Main loop: stream x, accumulate masked sums per example in PSUM.
        # (A/B'd small leading tiles for faster startup: no measurable win,
        # the extra tile-boundary waits on the PE stream cancel it.)
        def segments(b):
            return [chunks_per_tile] * (nchunk // chunks_per_tile)

        dma_i = 0
        for b in range(bs):
            # t = p*nchunk + n: per-partition reads are contiguous.
            x_b = x[b].rearrange("(p n) d -> p n d", p=P)  # [128, nchunk, d]
            acc_ps = accs.tile([1, d], mybir.dt.float32)
            n0 = 0
            for seg in segments(b):
                x_tile = xpool.tile([P, seg, d], mmdt, tag="x_tile")
                dma_engines[dma_i % len(dma_engines)].dma_start(
                    out=x_tile,
                    in_=x_b[:, n0 : n0 + seg, :],
                )
                dma_i += 1
                for k in range(seg):
                    n = n0 + k
                    nc.tensor.matmul(
                        acc_ps,
                        keep[:, b, n : n + 1],
                        x_tile[:, k, :],
                        start=(n == 0),
                        stop=(n == nchunk - 1),
                    )
                n0 += seg
            # out[b] = acc / denom[b]
            o_sb = tails.tile([1, d], mybir.dt.float32)
            nc.vector.tensor_scalar_mul(o_sb, acc_ps, rec[0:1, b : b + 1])
            out_dma.dma_start(out=out[b : b + 1, :], in_=o_sb)

    nc.finalize()
    return nc


def kernel(x: np.ndarray, mask: np.ndarray) -> np.ndarray:
    assert x.shape == (B, T, D) and mask.shape == (B, T)
    nc = build_bass()
    mask_u8 = np.ascontiguousarray(mask).view(np.uint8)
    in_maps = [
        {
            "x": np.ascontiguousarray(x[i * BS : (i + 1) * BS]),
            "mask": np.ascontiguousarray(mask_u8[i * BS : (i + 1) * BS]),
        }
        for i in range(N_CORES)
    ]
    res = bass_utils.run_bass_kernel_spmd(nc, in_maps, core_ids=list(range(N_CORES)))
    out = np.concatenate([r["out"] for r in res.results], axis=0)
    return out.astype(np.float32, copy=False)

